# revision 33
# baseline (speedup 1.0000x reference)
"""Trainium2 Bass kernel for nn_AttentionMatrix.

Computes, for mat_0:[B,N,H], mat_1:[B,M,H], w:[3H], bias:[1]:
    out[b,n,m] = sum_h mat_0[b,n,h]*w2[h]*mat_1[b,m,h] + s0[b,n] + s1[b,m] + C
with s0 = mat_0@w0, s1 = mat_1@w1, C = bias[0].

Strategy: data-parallel over batch across 8 NeuronCores (2 batches/core).
All rank-1/layout work happens on host; the device does only the batched
matmul + epilogue evictions.

Mixed-precision contraction: the host PERMUTES the h (contraction) axis by
|w2| and computes the 256 smallest-|w2| terms (7% of sum_h w2^2, so ~8e-3
rel-L2 error) in fp8e4m3 with DoubleRow (0.5 cycles/row - 2x PE rate),
and the 256 largest in bf16 (1.0). sqrt(|w2|) is split across both fp8
operands so values stay in e4m3's normal range. Per 128x512 psum region:
2 bf16 matmuls + 1 DoubleRow matmul = 1280 cycles vs 2048 all-bf16.

Epilogue: m-half-0 columns evict on DVE (psum + s0_col + s1_row fused);
m-half-1 columns evict on ACT as psum + s0 (ACT has no row-vector add;
the s1 row add for those columns happens on host, exactly, in f32). Both
engines stay under the 68us PE floor. bf16 stores; host upcasts.

Schedule (per core): warmup matmuls hide the PE clock ramp inside the
initial DMA window; batch-0 m-half-0 operands stream as k-interleaved
chunks; everything later is k-packed single DMAs; 8-deep ob pool decouples
evicts from store latency; the final tile drains via k-inner groups in
separate psum tiles with narrow evicts/stores on idle queues.
"""

import numpy as np

import concourse.bacc as bacc
import concourse.mybir as mybir
from concourse.tile import TileContext

F32 = mybir.dt.float32
BF16 = mybir.dt.bfloat16
FP8 = mybir.dt.float8e4
ADD = mybir.AluOpType.add
DROW = mybir.MatmulPerfMode.DoubleRow

P = 128

# Problem dims (hardcoded per contract)
B, N, M, H = 16, 2048, 2048, 512
N_CORES = 8
BPC = B // N_CORES  # batches per core

KB16 = 2            # bf16 k-tiles (256 largest-|w2| h dims)
WARMUPS = [256, 256, 256]  # PE ramp warmup matmul widths (f32)


def build_program(bpc=BPC, n=N, m=M, h=H):
    nt = n // P        # n-tiles (output partition tiles)
    hw_ = 1024         # half width (chunk/psum/store granularity)
    nh = m // hw_      # halves

    nc = bacc.Bacc("TRN2", target_bir_lowering=False, debug=False)
    # bf16 operands: [bpc, 256, n|m] (2 k-tiles, h-permuted)
    a_t = nc.dram_tensor("a_t", [bpc, KB16 * P, n], BF16,
                         kind="ExternalInput").ap()
    b_t = nc.dram_tensor("b_t", [bpc, KB16 * P, m], BF16,
                         kind="ExternalInput").ap()
    # fp8 operands: [bpc, 128, 2, n|m] (DoubleRow pair layout)
    a_8 = nc.dram_tensor("a_8", [bpc, P, 2, n], FP8,
                         kind="ExternalInput").ap()
    b_8 = nc.dram_tensor("b_8", [bpc, P, 2, m], FP8,
                         kind="ExternalInput").ap()
    # packed epilogue vectors: [:, 0:nt] = s0 columns, [:, nt:] = s1 row bcast
    svec = nc.dram_tensor("svec", [bpc, P, nt + m], BF16,
                          kind="ExternalInput").ap()
    out = nc.dram_tensor("out", [bpc, n, m], BF16, kind="ExternalOutput").ap()

    with TileContext(nc) as tc:
        with (
            tc.tile_pool(name="const", bufs=1) as cpool,
            tc.tile_pool(name="opnd", bufs=1) as tpool,
            tc.tile_pool(name="vecs", bufs=1) as vpool,
            tc.tile_pool(name="ob", bufs=8) as obpool,
            tc.tile_pool(name="mpsum", bufs=4, space="PSUM") as mpsum,
        ):
            # PE p-state warmup: dummy f32 matmuls (values never escape:
            # every real accumulation group starts with start=True) keep the
            # PE busy from ~t=0 so real matmuls start at full clock.
            zt = cpool.tile([P, 256], F32)
            nc.vector.memset(zt, 0.0)
            mpw = mpsum.tile([P, hw_], F32, tag="mm", name="mpw")
            for wu in WARMUPS:
                nc.tensor.matmul(
                    mpw[:, 0:wu],
                    lhsT=zt[:, 0:P],
                    rhs=zt[:, 0:wu],
                    start=True,
                    stop=True,
                )

            # ---- loads -------------------------------------------------
            # batch-0 h0: k-interleaved chunks (progressive head): bf16 k
            # pairs first (they start psum groups), fp8 pair after
            h0 = {}
            for k in range(KB16):
                for mat, src in (("b", b_t), ("a", a_t)):
                    t_ = tpool.tile([P, hw_], BF16, tag=f"{mat}{k}h0",
                                    name=f"{mat}{k}h0")
                    nc.sync.dma_start(
                        out=t_, in_=src[0, k * P:(k + 1) * P, 0:hw_]
                    )
                    h0[f"{mat}{k}"] = t_
            for mat, src in (("b", b_8), ("a", a_8)):
                t_ = tpool.tile([P, 2 * hw_], FP8, tag=f"{mat}8h0",
                                name=f"{mat}8h0")
                nc.sync.dma_start(
                    out=t_.rearrange("p (j w) -> p j w", j=2),
                    in_=src[0, :, :, 0:hw_],
                )
                h0[f"{mat}8"] = t_

            sv = {}
            sv[0] = vpool.tile([P, nt + m], BF16, tag="sv0", name="sv0")
            nc.sync.dma_start(out=sv[0], in_=svec[0])

            def load_pk16(bi, src, lo, hi, tag):
                """bf16 k-packed single DMA -> [P, 2, hi-lo] view."""
                w_ = hi - lo
                t_ = tpool.tile([P, KB16 * w_], BF16, tag=tag, name=tag)
                nc.sync.dma_start(
                    out=t_.rearrange("p (k w) -> p k w", k=KB16),
                    in_=src[bi, :, lo:hi].rearrange("(k p) w -> p k w", p=P),
                )
                return t_.rearrange("p (k w) -> p k w", k=KB16)

            def load_pk8(bi, src, lo, hi, tag):
                """fp8 DoubleRow-pair single DMA -> [P, 2, hi-lo] view."""
                w_ = hi - lo
                t_ = tpool.tile([P, 2 * w_], FP8, tag=tag, name=tag)
                nc.sync.dma_start(
                    out=t_.rearrange("p (j w) -> p j w", j=2),
                    in_=src[bi, :, :, lo:hi],
                )
                return t_.rearrange("p (j w) -> p j w", j=2)

            # batch-0 h1 halves, then batch-1 (all k-packed single DMAs)
            bh1_0 = load_pk16(0, b_t, hw_, m, "bh1_0")
            ah1_0 = load_pk16(0, a_t, hw_, m, "ah1_0")
            b8h1_0 = load_pk8(0, b_8, hw_, m, "b8h1_0")
            a8h1_0 = load_pk8(0, a_8, hw_, m, "a8h1_0")
            if bpc > 1:
                sv[1] = vpool.tile([P, nt + m], BF16, tag="sv1", name="sv1")
                nc.sync.dma_start(out=sv[1], in_=svec[1])
                bt1 = load_pk16(1, b_t, 0, m, "bt1")
                at1 = load_pk16(1, a_t, 0, n, "at1")
                b8_1 = load_pk8(1, b_8, 0, m, "b8_1")
                a8_1 = load_pk8(1, a_8, 0, n, "a8_1")

            # ---- compute ----------------------------------------------
            def emit_group(mp, lo, gw, lhs, rhs, lhs8, rhs8):
                """One psum 512-region: 2 bf16 matmuls + 1 fp8 DoubleRow."""
                for k in range(KB16):
                    nc.tensor.matmul(
                        mp[:, lo:lo + gw],
                        lhsT=lhs[k],
                        rhs=rhs[k][:, lo:lo + gw],
                        start=(k == 0),
                        stop=False,
                    )
                nc.tensor.matmul(
                    mp[:, lo:lo + gw],
                    lhsT=lhs8,
                    rhs=rhs8[:, :, lo:lo + gw],
                    start=False,
                    stop=True,
                    perf_mode=DROW,
                )

            def emit_tile(bi, t, hf, lhs, rhs, lhs8, rhs8, fine_tail=False):
                """One [128n, 1024m] output tile: matmuls + evict + store.

                lhs: k -> [P, P] bf16 lhsT AP; rhs: k -> [P, 1024] bf16 AP;
                lhs8: [P, 2, P] fp8 AP; rhs8: [P, 2, 1024] fp8 AP.
                hf 0: DVE stt evict (fused s1); hf 1: ACT psum+s0 evict
                (s1 added on host).
                """
                s0c = sv[bi][:, t:t + 1]
                s1o = nt + hf * hw_
                # evict engine alternates by (t+hf) parity so DVE and ACT
                # each take half the evicts in every emission phase; the
                # final tile is forced onto ACT (shorter drain chain). ACT
                # evicts are psum+s0 only - the host adds s1 there.
                on_act = (t + hf) % 2 == 1 or fine_tail
                if fine_tail:
                    # k-inner groups in separate psum tiles (a start-group
                    # WARs an in-flight evict of the same tile); narrow
                    # evicts + idle-queue stores drain the pipe fast
                    for gi, (glo, gw) in enumerate(fine_tail):
                        mp = mpsum.tile([P, hw_], F32, tag="mm", name="mp")
                        emit_group(mp, 0, gw,
                                   lhs,
                                   {k: rhs[k][:, glo:glo + gw]
                                    for k in range(KB16)},
                                   lhs8, rhs8[:, :, glo:glo + gw])
                        obc = obpool.tile([P, gw], BF16, tag=f"obf{gi}{hf}",
                                          name="obf", bufs=1)
                        nc.scalar.add(obc, mp[:, 0:gw], s0c)
                        nc.sync.dma_start(
                            out=out[bi, t * P:(t + 1) * P,
                                    hf * hw_ + glo:hf * hw_ + glo + gw],
                            in_=obc,
                        )
                    return
                mp = mpsum.tile([P, hw_], F32, tag="mm", name="mp")
                for mh in range(2):
                    emit_group(mp, mh * 512, 512, lhs, rhs, lhs8, rhs8)
                ob = obpool.tile([P, hw_], BF16, tag="ob", name="ob")
                if on_act:
                    nc.scalar.add(ob, mp, s0c)
                else:
                    nc.vector.scalar_tensor_tensor(
                        out=ob,
                        in0=mp,
                        scalar=s0c,
                        in1=sv[bi][:, s1o:s1o + hw_],
                        op0=ADD,
                        op1=ADD,
                    )
                nc.sync.dma_start(
                    out=out[bi, t * P:(t + 1) * P, hf * hw_:(hf + 1) * hw_],
                    in_=ob,
                )

            # batch 0: all h0 tiles first (h1 operands land later)
            for hf in range(nh):
                for t in range(nt):
                    if t < 8:
                        lhs = {
                            k: h0[f"a{k}"][:, t * P:(t + 1) * P]
                            for k in range(KB16)
                        }
                        lhs8 = h0["a8"].rearrange(
                            "p (j w) -> p j w", j=2
                        )[:, :, t * P:(t + 1) * P]
                    else:
                        lhs = {
                            k: ah1_0[:, k, (t - 8) * P:(t - 7) * P]
                            for k in range(KB16)
                        }
                        lhs8 = a8h1_0[:, :, (t - 8) * P:(t - 7) * P]
                    if hf == 0:
                        rhs = {k: h0[f"b{k}"] for k in range(KB16)}
                        rhs8 = h0["b8"].rearrange("p (j w) -> p j w", j=2)
                    else:
                        rhs = {k: bh1_0[:, k, :] for k in range(KB16)}
                        rhs8 = b8h1_0
                    emit_tile(0, t, hf, lhs, rhs, lhs8, rhs8)

            # batch 1
            if bpc > 1:
                for t in range(nt):
                    lhs = {
                        k: at1[:, k, t * P:(t + 1) * P] for k in range(KB16)
                    }
                    lhs8 = a8_1[:, :, t * P:(t + 1) * P]
                    for hf in range(nh):
                        rhs = {
                            k: bt1[:, k, hf * hw_:(hf + 1) * hw_]
                            for k in range(KB16)
                        }
                        rhs8 = b8_1[:, :, hf * hw_:(hf + 1) * hw_]
                        ft = False
                        if t == nt - 1:
                            # NOTE: matmul moving dim is ISA-capped at 512
                            ft = [(0, 512), (512, 512)]
                        emit_tile(1, t, hf, lhs, rhs, lhs8, rhs8,
                                  fine_tail=ft)
    nc.compile()
    return nc


_CACHE = {}


def _get_program():
    if "nc" not in _CACHE:
        _CACHE["nc"] = build_program()
    return _CACHE["nc"]


def make_in_maps(inputs, bpc=BPC, n_cores=N_CORES, n=N, m=M, h=H):
    import ml_dtypes

    bf16 = ml_dtypes.bfloat16
    fp8 = np.dtype(mybir.dt.np(FP8))
    mat_0 = np.asarray(inputs["mat_0"], dtype=np.float32)
    mat_1 = np.asarray(inputs["mat_1"], dtype=np.float32)
    w = np.asarray(inputs["w"], dtype=np.float32)
    bias = np.asarray(inputs["bias"], dtype=np.float32)
    w0, w1, w2 = w[:h], w[h:2 * h], w[2 * h:]
    nt = n // P
    # host-side rank-1 epilogue vectors (f32 compute, bf16 transport)
    s0 = mat_0 @ w0                      # [B, n]
    s1 = mat_1 @ w1 + bias[0]            # [B, m]
    # permute h by |w2|: largest 256 -> bf16 k-tiles, smallest 256 -> fp8
    perm = np.argsort(np.abs(w2))
    h8, hb = perm[:2 * P], perm[2 * P:]
    # bf16 side: w2 folded into a
    a_t = np.ascontiguousarray(
        (mat_0[:, :, hb] * w2[hb]).astype(bf16).transpose(0, 2, 1)
    )                                                   # [B, 256, n]
    b_t = np.ascontiguousarray(
        mat_1[:, :, hb].astype(bf16).transpose(0, 2, 1)  # [B, 256, m]
    )
    # fp8 side: sqrt(|w2|) split across both operands keeps values in
    # e4m3's normal range; sign goes to b. DoubleRow pair layout:
    # [B, 128, 2, n] with pair j = h8[j*128 + p]
    r = np.sqrt(np.abs(w2[h8]))
    a8v = (mat_0[:, :, h8] * r).astype(fp8).transpose(0, 2, 1)   # [B,256,n]
    b8v = (mat_1[:, :, h8] * (r * np.sign(w2[h8]))).astype(fp8)
    b8v = b8v.transpose(0, 2, 1)                                 # [B,256,m]
    a_8 = np.ascontiguousarray(
        a8v.reshape(-1, 2, P, n).transpose(0, 2, 1, 3))          # [B,128,2,n]
    b_8 = np.ascontiguousarray(
        b8v.reshape(-1, 2, P, m).transpose(0, 2, 1, 3))          # [B,128,2,m]
    s0t = s0.reshape(-1, nt, P).transpose(0, 2, 1)     # [B, P, nt]
    s1t = np.broadcast_to(s1[:, None, :], (s1.shape[0], P, m))  # [B, P, m]
    svec = np.ascontiguousarray(
        np.concatenate([s0t, s1t], axis=2)
    ).astype(bf16)                                     # [B, P, nt + m]
    in_maps = []
    for c in range(n_cores):
        sl = slice(c * bpc, (c + 1) * bpc)
        in_maps.append(
            {
                "a_t": a_t[sl],
                "b_t": b_t[sl],
                "a_8": a_8[sl],
                "b_8": b_8[sl],
                "svec": svec[sl],
            }
        )
    return in_maps, s1


def kernel(**inputs) -> np.ndarray:
    from concourse import bass_utils

    nc = _get_program()
    in_maps, s1 = make_in_maps(inputs)
    res = bass_utils.run_bass_kernel_spmd(
        nc, in_maps, core_ids=list(range(N_CORES))
    )
    full = np.concatenate(
        [np.asarray(res.results[c]["out"]) for c in range(N_CORES)], axis=0
    ).astype(np.float32)
    # ACT-evicted columns (m half 1) skip the s1 row add on device; apply
    # it here, exactly, in f32
    full[:, :, M // 2:] += s1[:, None, M // 2:]
    return full
